# revision 1
# baseline (speedup 1.0000x reference)
import numpy as np
import concourse.bass as bass
import concourse.mybir as mybir
from concourse import bacc
from concourse.tile import TileContext
from concourse.bass_utils import run_bass_kernel_spmd

B, N, D, K = 64, 1000, 1024, 256
NCORES = 8
BL = B // NCORES
SIGMA = 0.05
PT = 125
NT = N // PT

_nc_cache = {}


def _build():
    if "nc" in _nc_cache:
        return _nc_cache["nc"]
    nc = bacc.Bacc("TRN2", target_bir_lowering=False, debug=False,
                   num_devices=NCORES)
    x_ext = nc.dram_tensor("x", [BL, D], mybir.dt.float32,
                           kind="ExternalInput").ap()
    nz_ext = nc.dram_tensor("noise", [BL, N, D], mybir.dt.float32,
                            kind="ExternalInput").ap()
    idx_ext = nc.dram_tensor("idx", [BL, N, K], mybir.dt.uint16,
                             kind="ExternalOutput").ap()

    with TileContext(nc) as tc:
        with tc.tile_pool(name="xb", bufs=2) as xpool, \
             tc.tile_pool(name="work", bufs=3) as wpool, \
             tc.tile_pool(name="m8", bufs=3) as mpool, \
             tc.tile_pool(name="outp", bufs=3) as opool:
            for r in range(BL):
                xb = xpool.tile([128, D], mybir.dt.float32)
                nc.sync.dma_start(out=xb[:1, :], in_=x_ext[r:r + 1, :])
                nc.gpsimd.partition_broadcast(xb[:], xb[:1, :])
                for t in range(NT):
                    vals = wpool.tile([PT, D], mybir.dt.float32)
                    nc.sync.dma_start(out=vals[:],
                                      in_=nz_ext[r, t * PT:(t + 1) * PT, :])
                    nc.vector.scalar_tensor_tensor(
                        out=vals[:], in0=vals[:], scalar=SIGMA,
                        in1=xb[:PT, :],
                        op0=mybir.AluOpType.mult, op1=mybir.AluOpType.add)
                    m8 = mpool.tile([PT, 8], mybir.dt.float32)
                    i8 = opool.tile([PT, K], mybir.dt.uint16)
                    for j in range(K // 8):
                        nc.vector.max(m8[:], vals[:])
                        nc.vector.max_index(i8[:, j * 8:(j + 1) * 8],
                                            m8[:], vals[:])
                        nc.vector.match_replace(vals[:], m8[:], vals[:],
                                                -1e30)
                    nc.sync.dma_start(out=idx_ext[r, t * PT:(t + 1) * PT, :],
                                      in_=i8[:])
    nc.compile()
    _nc_cache["nc"] = nc
    return nc


def kernel(x, noise, k):
    nc = _build()
    in_maps = [
        {"x": np.ascontiguousarray(x[c * BL:(c + 1) * BL]),
         "noise": np.ascontiguousarray(noise[c * BL:(c + 1) * BL])}
        for c in range(NCORES)
    ]
    res = run_bass_kernel_spmd(nc, in_maps, core_ids=list(range(NCORES)))
    idx_all = np.concatenate(
        [res.results[c]["idx"][None] for c in range(NCORES)], axis=0)
    idx_all = idx_all.reshape(B, N, K)
    idx_sorted = np.sort(idx_all.astype(np.int64), axis=-1)
    rows = np.arange(B, dtype=np.int64)[:, None, None]
    kks = np.arange(K, dtype=np.int64)[None, None, :]
    bins = (rows * K + kks) * D + idx_sorted
    counts = np.bincount(bins.ravel(), minlength=B * K * D)
    return (counts.reshape(B, K, D) / float(N)).astype(np.float32)
